# revision 2
# baseline (speedup 1.0000x reference)
"""Single-head dense attention (B=4, S=2048, H=1024) on 8 TRN2 NeuronCores.

Sharding: data-parallel, core c -> (batch b = c//2, query-half h = c%2).
Each core receives its batch's x pre-transposed (xT [H, S], fp16, rolled so
its 1024 queries are always columns 0:1024 — attention is invariant to key
order), plus all four weights pre-transposed to [in, out] fp16.

Per-core pipeline (all matmuls fp16 inputs, fp32 PSUM accumulation, loops
ordered weight-stationary so each LDWEIGHTS is amortized over the matmuls
sharing that lhsT — measured ~2.5x cheaper than distinct-weight streams):
  1. K^T[e,ks], Q^T[e,q], V[ks,e] projections; chunked input DMAs so compute
     starts after 1/8 of the data arrives.
  2. Scores in natural [q, ks] layout per 128-row query chunk: each Q^T
     weight chunk amortized over 4 key tiles; exact softmax via per-row max
     (DVE reduce_max on the score PSUM) as a per-partition exp bias, exp to
     fp16 (safe: values <= 1), denominators free via ACT accum_out. The
     transpose AND the normalization are fused into one regular fp16 matmul
     per 128x128 tile: P^T = exp^T @ diag(1/denom).
  3. out^T[e,q] = V.T @ P^T with each V weight chunk shared across both
     q-tiles; y = out^T.T @ Wo^T with each out^T chunk shared across both
     f-tiles. All PSUM pools coexist (no pool-close barriers stalling PE).
"""

from contextlib import ExitStack

import numpy as np

import bass_rust
import concourse.bass as bass
import concourse.mybir as mybir
import concourse.tile as tile
from concourse.vector_clock import ScopedClock
from concourse.masks import make_identity

HIDDEN = 1024
BATCH, SEQ = 4, 2048
P = 128
QH = 1024  # queries per core
NCORES = 8

F16 = mybir.dt.float16
F32 = mybir.dt.float32

# ---------------------------------------------------------------------------
# Workaround: walrus in this container encodes a limited number of sync-wait
# commands per instruction (1 for Matmult/Ldweights, ~4 for control insts).
# Split overflow waits onto same-engine NOPs inserted before the instruction,
# and split the Tile tail-drain waits onto sync-engine NOPs.
# ---------------------------------------------------------------------------
_MAXW = 1
_nop_ctr = [0]


def _patched_drain_and_barrier(self, tick_clock, wait_clock):
    nc = self.nc
    drain_inst = nc.sync.drain()
    wait_clock.add_sem_waits(
        drain_inst.ins, ScopedClock({None: tick_clock.global_clock})
    )
    si = drain_inst.ins.sync_info
    if si is not None and len(si.on_wait) > _MAXW:
        waits = list(si.on_wait)
        drain_inst.ins.sync_info = bass_rust.SyncInfo(
            on_wait=waits[:_MAXW], on_update=list(si.on_update)
        )
        for i in range(_MAXW, len(waits), _MAXW):
            nop = nc.sync.nop()
            nop.ins.sync_info = bass_rust.SyncInfo(
                on_wait=waits[i : i + _MAXW], on_update=[]
            )
    nc.all_engine_barrier()
    assert self.sems is not None
    popped = nc._tile_sem_poison_stack.pop()
    assert popped is self._sem_poison
    nc.clear_and_free_semaphores(list(self.sems.allocated().values()))
    nc.all_engine_barrier()


tile.TileContext._drain_and_barrier = _patched_drain_and_barrier


def _split_all_instruction_waits(nc):
    for f in nc.m.functions:
        for b in f.blocks:
            insts = b.instructions
            new_list = []
            changed = False
            for inst in insts:
                si = getattr(inst, "sync_info", None)
                if si is not None and len(si.on_wait) > _MAXW:
                    waits = list(si.on_wait)
                    keep = waits[-_MAXW:]
                    overflow = waits[:-_MAXW]
                    for j in range(0, len(overflow), _MAXW):
                        _nop_ctr[0] += 1
                        nop = mybir.InstNoOp(
                            name=f"I-waitsplit-{_nop_ctr[0]}",
                            engine=inst.engine,
                            bass_nofuse=True,
                            sync_info=bass_rust.SyncInfo(
                                on_wait=overflow[j : j + _MAXW], on_update=[]
                            ),
                        )
                        new_list.append(nop)
                    inst.sync_info = bass_rust.SyncInfo(
                        on_wait=keep, on_update=list(si.on_update)
                    )
                    changed = True
                new_list.append(inst)
            if changed:
                insts[:] = new_list


# ---------------------------------------------------------------------------
# Kernel program
# ---------------------------------------------------------------------------

def _build_program(repeat=1, tweak=0):
    nc = bass.Bass(
        "TRN2", target_bir_lowering=False, debug=False, num_devices=NCORES
    )
    xt_d = nc.dram_tensor("xt", [HIDDEN, SEQ], F16, kind="ExternalInput").ap()
    wqt_d = nc.dram_tensor("wqt", [HIDDEN, HIDDEN], F16, kind="ExternalInput").ap()
    wkt_d = nc.dram_tensor("wkt", [HIDDEN, HIDDEN], F16, kind="ExternalInput").ap()
    wvt_d = nc.dram_tensor("wvt", [HIDDEN, HIDDEN], F16, kind="ExternalInput").ap()
    wot_d = nc.dram_tensor("wot", [HIDDEN, HIDDEN], F16, kind="ExternalInput").ap()
    y_d = nc.dram_tensor("y", [QH, HIDDEN], F32, kind="ExternalOutput").ap()

    HO = HIDDEN // P  # 8 chunks of the hidden/feature dim
    SO = SEQ // P  # 16 chunks of the key seq dim

    with tile.TileContext(nc) as tc:
      for _t in range(tweak):
        nc.sync.nop()
      for _rep in range(repeat):
        with ExitStack() as ctx:
            _kernel_body(nc, tc, ctx, xt_d, wqt_d, wkt_d, wvt_d, wot_d, y_d, HO, SO)

    _split_all_instruction_waits(nc)
    return nc


def _kernel_body(nc, tc, ctx, xt_d, wqt_d, wkt_d, wvt_d, wot_d, y_d, HO, SO):
    if True:
        pers = ctx.enter_context(tc.tile_pool(name="pers", bufs=1))
        kt_sb = pers.tile([P, HO, SEQ], F16)  # K^T [e, ks]
        qt_sb = pers.tile([P, HO, QH], F16)  # Q^T [e, q]
        v_sb = pers.tile([P, SO, HIDDEN], F16)  # V [ks, e]
        wot_sb = pers.tile([P, HO, HIDDEN], F16)  # Wo^T [e, f]

        # ---- Phase 1: projections (weight-stationary loop order: each
        # LDWEIGHTS is amortized over all matmuls sharing that lhsT) ----
        with ExitStack() as p1:
            proj = p1.enter_context(tc.tile_pool(name="proj", bufs=1))
            xt_sb = proj.tile([P, HO, SEQ], F16)
            wq_sb = proj.tile([P, HO, HIDDEN], F16, tag="wq")
            wk_sb = proj.tile([P, HO, HIDDEN], F16, tag="wk")
            wv_sb = proj.tile([P, HO, HIDDEN], F16, tag="wv")
            # Chunked input DMAs, ordered so KT's first accumulation chunk
            # (wk[0], xt[0]) lands first and compute starts immediately.
            xt_r = xt_d.rearrange("(o p) s -> p o s", p=P)
            wq_r = wqt_d.rearrange("(o p) e -> p o e", p=P)
            wk_r = wkt_d.rearrange("(o p) e -> p o e", p=P)
            wv_r = wvt_d.rearrange("(o p) e -> p o e", p=P)
            nc.sync.dma_start(wk_sb[:, 0, 0:512], wk_r[:, 0, 0:512])
            nc.sync.dma_start(xt_sb[:, 0, 0:512], xt_r[:, 0, 0:512])
            nc.sync.dma_start(wk_sb[:, 0, 512:1024], wk_r[:, 0, 512:1024])
            nc.sync.dma_start(xt_sb[:, 0, 512:2048], xt_r[:, 0, 512:2048])
            for dc in range(1, HO):
                nc.sync.dma_start(wk_sb[:, dc], wk_r[:, dc])
                nc.sync.dma_start(xt_sb[:, dc], xt_r[:, dc])
            for dc in range(HO):
                nc.sync.dma_start(wq_sb[:, dc], wq_r[:, dc])
            for dc in range(HO):
                nc.sync.dma_start(wv_sb[:, dc], wv_r[:, dc])
            nc.sync.dma_start(wot_sb[:], wot_d.rearrange("(o p) f -> p o f", p=P))

            # One projection PSUM pool (single tag, 4 banks/slot, 2 slots) so
            # the KT -> QT -> V phases flow without pool-close barriers.
            ps_proj = p1.enter_context(
                tc.tile_pool(name="ps_proj", bufs=2, space="PSUM")
            )
            # K^T[e, ks] = sum_d WkT[d, e] * xT[d, ks]
            for ec in range(HO):
                pt = ps_proj.tile([P, SEQ // 512, 512], F32, tag="proj")
                for dc in range(HO):
                    for st in range(SEQ // 512):
                        nc.tensor.matmul(
                            pt[:, st],
                            lhsT=wk_sb[:, dc, ec * P : (ec + 1) * P],
                            rhs=xt_sb[:, dc, st * 512 : (st + 1) * 512],
                            start=(dc == 0),
                            stop=(dc == HO - 1),
                        )
                nc.vector.tensor_copy(out=kt_sb[:, ec], in_=pt[:])
            # Q^T[e, q] (queries are xT columns 0:QH)
            for ec in range(HO):
                pt = ps_proj.tile([P, SEQ // 512, 512], F32, tag="proj", name="pt_q")
                for dc in range(HO):
                    for st in range(QH // 512):
                        nc.tensor.matmul(
                            pt[:, st],
                            lhsT=wq_sb[:, dc, ec * P : (ec + 1) * P],
                            rhs=xt_sb[:, dc, st * 512 : (st + 1) * 512],
                            start=(dc == 0),
                            stop=(dc == HO - 1),
                        )
                nc.vector.tensor_copy(
                    out=qt_sb[:, ec], in_=pt[:, : QH // 512]
                )
            # V[ks, e] = sum_d xT[d, ks] * WvT[d, e]
            for sc in range(SO):
                pt = ps_proj.tile([P, SEQ // 512, 512], F32, tag="proj", name="pt_v")
                for dc in range(HO):
                    for et in range(HIDDEN // 512):
                        nc.tensor.matmul(
                            pt[:, et],
                            lhsT=xt_sb[:, dc, sc * P : (sc + 1) * P],
                            rhs=wv_sb[:, dc, et * 512 : (et + 1) * 512],
                            start=(dc == 0),
                            stop=(dc == HO - 1),
                        )
                nc.vector.tensor_copy(
                    out=v_sb[:, sc], in_=pt[:, : HIDDEN // 512]
                )

        # ---- Phase 2/3: attention + output projection ----
        # Scores in natural [q, ks] layout: each Q^T weight chunk is loaded
        # once and amortized over 4 key tiles; softmax denominators come free
        # from ACT accum_out (per-partition), normalize is a per-partition
        # tensor_scalar; P-hat is then PE-transposed (fp16, packed 8 tiles
        # per PSUM bank) into the [ks, q] layout PV needs.
        pers2 = ctx.enter_context(tc.tile_pool(name="pers2", bufs=1))
        phat_sb = pers2.tile([P, SO, QH], F16)  # P^T [ks, q], both q-tiles
        outt_sb = pers2.tile([P, HO, QH], F16)  # out^T [e, q]
        ident = pers2.tile([P, P], F16)
        make_identity(nc, ident[:])
        attb = ctx.enter_context(tc.tile_pool(name="attb", bufs=2))
        smallb = ctx.enter_context(tc.tile_pool(name="smallb", bufs=6))
        ystage = ctx.enter_context(tc.tile_pool(name="ystage", bufs=2))
        ps_sc = ctx.enter_context(tc.tile_pool(name="ps_sc", bufs=3, space="PSUM"))
        ps_tp = ctx.enter_context(tc.tile_pool(name="ps_tp", bufs=2, space="PSUM"))

        NQ = QH // P  # 8 query chunks of 128
        for qc in range(NQ):
            exp_sb = attb.tile([P, 4, 512], F16, tag="expP", name="exp_sb")
            sc_tiles = []
            for kh in range(2):
                sc_ps = ps_sc.tile([P, 2, 512], F32, tag="sc", name="sc_ps")
                sc_tiles.append(sc_ps)
                for ec in range(HO):
                    for kst in range(2):
                        nc.tensor.matmul(
                            sc_ps[:, kst],
                            lhsT=qt_sb[:, ec, qc * P : (qc + 1) * P],
                            rhs=kt_sb[:, ec, (kh * 2 + kst) * 512 : (kh * 2 + kst + 1) * 512],
                            start=(ec == 0),
                            stop=(ec == HO - 1),
                        )
            # exact per-row max -> negated bias for exp
            m0 = smallb.tile([P, 1], F32, tag="m0", name="m0")
            m1 = smallb.tile([P, 1], F32, tag="m1", name="m1")
            nc.vector.reduce_max(m0[:], sc_tiles[0][:], axis=mybir.AxisListType.XY)
            nc.vector.reduce_max(m1[:], sc_tiles[1][:], axis=mybir.AxisListType.XY)
            negmax = smallb.tile([P, 1], F32, tag="negmax", name="negmax")
            nc.vector.tensor_tensor(
                negmax[:], m0[:], m1[:], mybir.AluOpType.max
            )
            nc.vector.tensor_scalar_mul(negmax[:], negmax[:], -1.0)
            accs = []
            for kh in range(2):
                for kst in range(2):
                    acc = smallb.tile([P, 1], F32, tag="acc", name="acc")
                    nc.scalar.activation(
                        exp_sb[:, kh * 2 + kst],
                        sc_tiles[kh][:, kst],
                        mybir.ActivationFunctionType.Exp,
                        bias=negmax[:],
                        scale=1.0,
                        accum_out=acc[:],
                    )
                    accs.append(acc)
            den = smallb.tile([P, 1], F32, tag="den", name="den")
            nc.vector.tensor_add(out=den[:], in0=accs[0][:], in1=accs[1][:])
            nc.vector.tensor_add(out=den[:], in0=den[:], in1=accs[2][:])
            nc.vector.tensor_add(out=den[:], in0=den[:], in1=accs[3][:])
            rec = smallb.tile([P, 1], F32, tag="rec", name="rec")
            nc.vector.reciprocal(rec[:], den[:])
            # transpose + normalize in one regular matmul per 128x128 tile:
            # P^T[ks, q] = sum_q' exp[q', ks] * diag(recip)[q', q]
            diag = smallb.tile([P, P], F16, tag="diag", name="diag")
            nc.vector.tensor_scalar_mul(diag[:], ident[:], rec[:])
            for half in range(4):
                tp_ps = ps_tp.tile([P, 4, P], F32, tag="tp", name="tp_ps")
                for k4 in range(4):
                    ksc = half * 4 + k4
                    nc.tensor.matmul(
                        tp_ps[:, k4],
                        lhsT=exp_sb[:, ksc // 4, (ksc % 4) * P : (ksc % 4 + 1) * P],
                        rhs=diag[:],
                        start=(k4 == 0),
                        stop=(k4 == 3),
                    )
                nc.vector.tensor_copy(
                    out=phat_sb[:, half * 4 : (half + 1) * 4, qc * P : (qc + 1) * P],
                    in_=tp_ps[:],
                )
        # PV: out^T[e, q] = sum_ks V[ks, e] * P^T[ks, q]; both q-tiles share
        # each V weight load; PSUM slots reuse the score tag.
        for half in range(4):
            pv_ps0 = ps_sc.tile([P, 2, 512], F32, tag="sc", name="pv_ps0")
            pv_ps1 = ps_sc.tile([P, 2, 512], F32, tag="sc", name="pv_ps1")
            for ks in range(SO):
                for e2 in range(2):
                    ec = half * 2 + e2
                    for qt, pv_ps in ((0, pv_ps0), (1, pv_ps1)):
                        nc.tensor.matmul(
                            pv_ps[:, e2],
                            lhsT=v_sb[:, ks, ec * P : (ec + 1) * P],
                            rhs=phat_sb[:, ks, qt * 512 : (qt + 1) * 512],
                            start=(ks == 0),
                            stop=(ks == SO - 1),
                        )
            for e2 in range(2):
                nc.vector.tensor_copy(
                    out=outt_sb[:, half * 2 + e2, 0:512], in_=pv_ps0[:, e2]
                )
                nc.scalar.copy(
                    out=outt_sb[:, half * 2 + e2, 512:1024], in_=pv_ps1[:, e2]
                )
        # y[sq, f] = sum_e outT[e, sq] * WoT[e, f]; both f-tiles share each
        # outT weight load; PSUM reuses the score tag.
        for sqc in range(QH // P):
            sq0 = sqc * P
            y_ps = ps_sc.tile([P, 2, 512], F32, tag="sc", name="y_ps")
            for ec in range(HO):
                for ft in range(HIDDEN // 512):
                    nc.tensor.matmul(
                        y_ps[:, ft],
                        lhsT=outt_sb[:, ec, sq0 : sq0 + P],
                        rhs=wot_sb[:, ec, ft * 512 : (ft + 1) * 512],
                        start=(ec == 0),
                        stop=(ec == HO - 1),
                    )
            y_sb = ystage.tile([P, 2, 512], F32, tag="ystage", name="y_sb")
            nc.vector.tensor_copy(out=y_sb[:, 0], in_=y_ps[:, 0])
            nc.scalar.copy(out=y_sb[:, 1], in_=y_ps[:, 1])
            nc.sync.dma_start(y_d[sq0 : sq0 + P, 0:512], y_sb[:, 0])
            nc.sync.dma_start(y_d[sq0 : sq0 + P, 512:1024], y_sb[:, 1])


_cached_nc = None


def prep_in_maps(x, Wq, Wk, Wv, Wo):
    x = np.asarray(x, dtype=np.float32)
    Wq = np.asarray(Wq, dtype=np.float32)
    Wk = np.asarray(Wk, dtype=np.float32)
    Wv = np.asarray(Wv, dtype=np.float32)
    Wo = np.asarray(Wo, dtype=np.float32)

    # Host-side layout prep (no FLOPs): transpose weights to [in, out], cast fp16.
    wqt = np.ascontiguousarray(Wq.T).astype(np.float16)
    wkt = np.ascontiguousarray(Wk.T).astype(np.float16)
    wvt = np.ascontiguousarray(Wv.T).astype(np.float16)
    wot = np.ascontiguousarray(Wo.T).astype(np.float16)

    in_maps = []
    for c in range(NCORES):
        b, h = divmod(c, 2)
        qlo = h * QH
        xt = np.ascontiguousarray(x[b].T)  # [H, S]
        # roll keys so this core's queries are always columns 0:QH
        xt_roll = np.concatenate([xt[:, qlo:], xt[:, :qlo]], axis=1).astype(
            np.float16
        )
        in_maps.append(
            {"xt": xt_roll, "wqt": wqt, "wkt": wkt, "wvt": wvt, "wot": wot}
        )
    return in_maps


def kernel(x, Wq, Wk, Wv, Wo):
    global _cached_nc
    from concourse.bass_utils import run_bass_kernel_spmd

    in_maps = prep_in_maps(x, Wq, Wk, Wv, Wo)

    if _cached_nc is None:
        _cached_nc = _build_program()
    res = run_bass_kernel_spmd(_cached_nc, in_maps, core_ids=list(range(NCORES)))

    out = np.empty((BATCH, SEQ, HIDDEN), dtype=np.float32)
    for c in range(NCORES):
        b, h = divmod(c, 2)
        out[b, h * QH : (h + 1) * QH, :] = res.results[c]["y"]
    return out



# revision 33
# speedup vs baseline: 18.5620x; 18.5620x over previous
"""Single-head dense attention (B=4, S=2048, H=1024) on 8 TRN2 NeuronCores.

Sharding: data-parallel, core c -> (batch b = c//2, query-half h = c%2).
Each core receives its batch's x in BOTH layouts (xT [H, S] and x [S, H],
fp16, rolled so its 1024 queries are always columns/rows 0:1024 — attention
is invariant to key order), plus Wq, Wk, Wv as stored [out, in] and Wo^T.

Weight-product reformulation: the reference is
    y = softmax(x Wq^T Wk x^T) x Wv^T Wo^T
so the four weights only enter through M = Wq^T Wk and N = Wv^T Wo^T
(both [H, H]). Computing M, N on device (2.15 GFLOP each) replaces the
K-projection and V-projection (4.3 GFLOP each at S=2048), cutting per-core
PE work from ~22.6 to ~17.7 GFLOP. Every matmul's lhsT/rhs falls in natural
layout: M <- (Wq, Wk), N <- (Wv, Wo^T), tT <- (M, xT), S <- (tT, xT),
U^T <- (x, P^T), y <- (U^T, N).

Per-core pipeline (fp16 inputs, fp32 PSUM accumulation; loops ordered so
each LDWEIGHTS is amortized over all matmuls sharing that lhsT):
  A. M[d,d'] and N[d,f]; chunked weight DMAs so compute starts early; Wv/WoT
     stream into the buffers Wq/Wk vacated (phase B covers the reload).
  B. tT[d',q] = sum_d M[d,d'] xT[d,q] over the core's 1024 query columns.
  C. Scores per 128-query chunk in natural [q,k] layout, each tT weight
     chunk amortized over 4 key tiles; exact softmax via per-row max (DVE
     reduce_max on score PSUM) as exp bias, exp to fp16, denominators free
     via ACT accum_out. The [q,k]->[k,q] transpose runs entirely on the DMA
     engines (XBAR transpose, one DMA per query chunk) — zero PE work.
  D. U^T[d,q] = sum_k x[k,d] P^T_raw[k,q] (P^T left unnormalized); the
     first two d-chunks are split by query half so their low-half matmuls
     cover the last chunk's softmax+transpose latency.
  E. y[q,f] = sum_d U^T[d,q] N[d,f]; softmax 1/denom folded into the
     PSUM->SBUF copies as a per-partition scale; fp16 y DMA'd on the
     second HWDGE queue.

Scheduling: all tile pools persist for the whole program (no alloc/release
boundaries between phases or repeats -> no PSUM/SBUF address-reuse stalls);
one shared 4-slot PSUM ring serves every accumulation phase; the next
repeat's input DMAs preload during this repeat's attention phases.
"""

from contextlib import ExitStack

import numpy as np

import bass_rust
import concourse.bass as bass
import concourse.mybir as mybir
import concourse.tile as tile
from concourse.vector_clock import ScopedClock
from concourse.masks import make_identity

HIDDEN = 1024
BATCH, SEQ = 4, 2048
P = 128
QH = 1024  # queries per core
NCORES = 8

F16 = mybir.dt.float16
F32 = mybir.dt.float32

# ---------------------------------------------------------------------------
# Workaround: walrus in this container encodes a limited number of sync-wait
# commands per instruction (1 for Matmult/Ldweights, ~4 for control insts).
# Split overflow waits onto same-engine NOPs inserted before the instruction,
# and split the Tile tail-drain waits onto sync-engine NOPs.
# ---------------------------------------------------------------------------
_MAXW = 1
_nop_ctr = [0]


def _patched_drain_and_barrier(self, tick_clock, wait_clock):
    nc = self.nc
    drain_inst = nc.sync.drain()
    wait_clock.add_sem_waits(
        drain_inst.ins, ScopedClock({None: tick_clock.global_clock})
    )
    si = drain_inst.ins.sync_info
    if si is not None and len(si.on_wait) > _MAXW:
        waits = list(si.on_wait)
        drain_inst.ins.sync_info = bass_rust.SyncInfo(
            on_wait=waits[:_MAXW], on_update=list(si.on_update)
        )
        for i in range(_MAXW, len(waits), _MAXW):
            nop = nc.sync.nop()
            nop.ins.sync_info = bass_rust.SyncInfo(
                on_wait=waits[i : i + _MAXW], on_update=[]
            )
    nc.all_engine_barrier()
    assert self.sems is not None
    popped = nc._tile_sem_poison_stack.pop()
    assert popped is self._sem_poison
    nc.clear_and_free_semaphores(list(self.sems.allocated().values()))
    nc.all_engine_barrier()


tile.TileContext._drain_and_barrier = _patched_drain_and_barrier


def _split_all_instruction_waits(nc):
    for f in nc.m.functions:
        for b in f.blocks:
            insts = b.instructions
            new_list = []
            changed = False
            for inst in insts:
                si = getattr(inst, "sync_info", None)
                if si is not None and len(si.on_wait) > _MAXW:
                    waits = list(si.on_wait)
                    keep = waits[-_MAXW:]
                    overflow = waits[:-_MAXW]
                    for j in range(0, len(overflow), _MAXW):
                        _nop_ctr[0] += 1
                        nop = mybir.InstNoOp(
                            name=f"I-waitsplit-{_nop_ctr[0]}",
                            engine=inst.engine,
                            bass_nofuse=True,
                            sync_info=bass_rust.SyncInfo(
                                on_wait=overflow[j : j + _MAXW], on_update=[]
                            ),
                        )
                        new_list.append(nop)
                    inst.sync_info = bass_rust.SyncInfo(
                        on_wait=keep, on_update=list(si.on_update)
                    )
                    changed = True
                new_list.append(inst)
            if changed:
                insts[:] = new_list


# ---------------------------------------------------------------------------
# Kernel program
# ---------------------------------------------------------------------------

def _build_program(repeat=1, tweak=0):
    nc = bass.Bass(
        "TRN2", target_bir_lowering=False, debug=False, num_devices=NCORES
    )
    xt_d = nc.dram_tensor("xt", [HIDDEN, SEQ], F16, kind="ExternalInput").ap()
    xn_d = nc.dram_tensor("xn", [SEQ, HIDDEN], F16, kind="ExternalInput").ap()
    wq_d = nc.dram_tensor("wq", [HIDDEN, HIDDEN], F16, kind="ExternalInput").ap()
    wk_d = nc.dram_tensor("wk", [HIDDEN, HIDDEN], F16, kind="ExternalInput").ap()
    wv_d = nc.dram_tensor("wv", [HIDDEN, HIDDEN], F16, kind="ExternalInput").ap()
    wot_d = nc.dram_tensor("wot", [HIDDEN, HIDDEN], F16, kind="ExternalInput").ap()
    y_d = nc.dram_tensor("y", [QH, HIDDEN], F16, kind="ExternalOutput").ap()

    HO = HIDDEN // P  # 8 chunks of the hidden dim
    SO = SEQ // P  # 16 chunks of the key seq dim

    with tile.TileContext(nc) as tc:
      for _t in range(tweak):
        nc.sync.nop()
      with ExitStack() as ctx:
        # All pools live for the whole program (every repeat): no pool
        # alloc/release boundaries between phases or repeats, so no PSUM /
        # SBUF address-reuse stalls at transitions.  PSUM: one shared ring
        # of 3x [P,2,512] slots (6 banks) for every accumulation phase +
        # 2x 1-bank transpose slots = 8 banks exactly.
        pools = {
            "pers": ctx.enter_context(tc.tile_pool(name="pers", bufs=1)),
            "pers2": ctx.enter_context(tc.tile_pool(name="pers2", bufs=1)),
            "attb": ctx.enter_context(tc.tile_pool(name="attb", bufs=2)),
            "smallb": ctx.enter_context(tc.tile_pool(name="smallb", bufs=6)),
            "ystage": ctx.enter_context(tc.tile_pool(name="ystage", bufs=2)),
            "wpool": ctx.enter_context(tc.tile_pool(name="wpool", bufs=1)),
            "ps": ctx.enter_context(tc.tile_pool(name="ps", bufs=4, space="PSUM")),
        }
        for _rep in range(repeat):
            _kernel_body(
                nc, tc, pools, xt_d, xn_d, wq_d, wk_d, wv_d, wot_d, y_d, HO, SO
            )

    _split_all_instruction_waits(nc)
    return nc


def _kernel_body(nc, tc, pools, xt_d, xn_d, wq_d, wk_d, wv_d, wot_d, y_d, HO, SO):
    pers = pools["pers"]
    xt_sb = pers.tile([P, HO, SEQ], F16, tag="xt")  # x^T [d, s] (queries 0:QH)
    xn_sb = pers.tile([P, SO, HIDDEN], F16, tag="xn")  # x [s, d]
    m_sb = pers.tile([P, HO, HIDDEN], F16, tag="m")  # M [d, d']
    n_sb = pers.tile([P, HO, HIDDEN], F16, tag="n")  # N [d, f]
    tt_sb = pers.tile([P, HO, QH], F16, tag="tt")  # t^T [d', q]
    pers2 = pools["pers2"]
    phat_sb = pers2.tile([P, SO, QH], F16, tag="phat")  # P^T [k, q] (raw exp)
    ut_sb = pers2.tile([P, HO, QH], F16, tag="ut")  # U^T [d, q] (unnormalized)
    rec_sb = pers2.tile([P, QH // P], F32, tag="recs")  # 1/denom per q chunk
    attb = pools["attb"]
    smallb = pools["smallb"]
    ystage = pools["ystage"]
    ps_mm = pools["ps"]

    # ---- Phase A/B: weight products M, N and t^T ----
    # Two weight buffers only: M consumes (Wq, Wk); afterwards Wv/WoT are
    # DMA'd into the same buffers for N (phase B covers that reload).
    wa_sb = pools["wpool"].tile([P, HO, HIDDEN], F16, tag="wa")
    wb_sb = pools["wpool"].tile([P, HO, HIDDEN], F16, tag="wb")
    wq_r = wq_d.rearrange("(o p) d -> p o d", p=P)
    wk_r = wk_d.rearrange("(o p) d -> p o d", p=P)
    wv_r = wv_d.rearrange("(o p) d -> p o d", p=P)
    wot_r = wot_d.rearrange("(o p) f -> p o f", p=P)
    xt_r = xt_d.rearrange("(o p) s -> p o s", p=P)
    xn_r = xn_d.rearrange("(o p) d -> p o d", p=P)
    # M's first PSUM group (dt=0) needs wq[:, :, 0:128] and wk chunk by
    # chunk — land those first so PE starts after ~0.5 MB of DMA.
    nc.sync.dma_start(wa_sb[:, :, 0:P], wq_r[:, :, 0:P])
    for dc in range(HO):
        nc.sync.dma_start(wb_sb[:, dc], wk_r[:, dc])
    for dt in range(1, HO):
        nc.sync.dma_start(
            wa_sb[:, :, dt * P : (dt + 1) * P], wq_r[:, :, dt * P : (dt + 1) * P]
        )
    for dc in range(HO):
        nc.sync.dma_start(xt_sb[:, dc, 0:QH], xt_r[:, dc, 0:QH])

    # M[d, d'] = sum_e Wq[e, d] Wk[e, d']
    for dt in range(HO):
        pt = ps_mm.tile([P, 2, 512], F32, tag="mm", name="pt_m")
        for ec in range(HO):
            for h in range(2):
                nc.tensor.matmul(
                    pt[:, h],
                    lhsT=wa_sb[:, ec, dt * P : (dt + 1) * P],
                    rhs=wb_sb[:, ec, h * 512 : (h + 1) * 512],
                    start=(ec == 0),
                    stop=(ec == HO - 1),
                )
        nc.vector.tensor_copy(out=m_sb[:, dt, 0:512], in_=pt[:, 0])
        nc.scalar.copy(out=m_sb[:, dt, 512:1024], in_=pt[:, 1])
    # Wv/WoT reload into the same buffers — emitted AFTER M's matmuls
    # (Tile dataflow is program-order), lands while B runs.
    for dc in range(HO):
        nc.sync.dma_start(wa_sb[:, dc], wv_r[:, dc])
        nc.sync.dma_start(wb_sb[:, dc], wot_r[:, dc])
    for dc in range(HO):
        nc.sync.dma_start(xt_sb[:, dc, QH:SEQ], xt_r[:, dc, QH:SEQ])
    for sc in range(SO):
        nc.sync.dma_start(xn_sb[:, sc], xn_r[:, sc])
    # t^T[d', q] = sum_d M[d, d'] xT[d, q]  (queries only).  Runs while
    # Wv/WoT stream into the weight buffers.
    for dpt in range(HO):
        pt = ps_mm.tile([P, 2, 512], F32, tag="mm", name="pt_t")
        for dc in range(HO):
            for h in range(2):
                nc.tensor.matmul(
                    pt[:, h],
                    lhsT=m_sb[:, dc, dpt * P : (dpt + 1) * P],
                    rhs=xt_sb[:, dc, h * 512 : (h + 1) * 512],
                    start=(dc == 0),
                    stop=(dc == HO - 1),
                )
        nc.vector.tensor_copy(out=tt_sb[:, dpt, 0:512], in_=pt[:, 0])
        nc.scalar.copy(out=tt_sb[:, dpt, 512:1024], in_=pt[:, 1])
    # N[d, f] = sum_e Wv[e, d] WoT[e, f] — also covers the tail latency
    # of tt_sb's last PSUM->SBUF copy before C.
    for dt in range(HO):
        pt = ps_mm.tile([P, 2, 512], F32, tag="mm", name="pt_n")
        for ec in range(HO):
            for h in range(2):
                nc.tensor.matmul(
                    pt[:, h],
                    lhsT=wa_sb[:, ec, dt * P : (dt + 1) * P],
                    rhs=wb_sb[:, ec, h * 512 : (h + 1) * 512],
                    start=(ec == 0),
                    stop=(ec == HO - 1),
                )
        nc.vector.tensor_copy(out=n_sb[:, dt, 0:512], in_=pt[:, 0])
        nc.scalar.copy(out=n_sb[:, dt, 512:1024], in_=pt[:, 1])

    # ---- Phase C: scores + softmax + XBAR-DMA transpose ----
    # The transpose off-loads to the DMA engines entirely: one XBAR DMA per
    # query chunk writes phat[:, ks, qc*P+p] = exp[q, ks*128+p] (verified
    # semantics).  Normalization (1/denom) is folded into phase E's
    # PSUM->SBUF copies as a per-partition scale, so PE does no transpose
    # work at all.
    ps_sc = ps_mm

    NQ = QH // P  # 8 query chunks of 128
    for qc in range(NQ):
        exp_sb = attb.tile([P, 4, 512], F16, tag="expP", name="exp_sb")
        sc0 = ps_sc.tile([P, 2, 512], F32, tag="mm", name="sc_ps0")
        sc1 = ps_sc.tile([P, 2, 512], F32, tag="mm", name="sc_ps1")
        sc_tiles = [sc0, sc1]
        # accumulation inner over dpc: each tT weight chunk amortized over
        # all 4 key tiles (one LDWEIGHTS per dpc)
        for dpc in range(HO):
            for kh in range(2):
                for kst in range(2):
                    nc.tensor.matmul(
                        sc_tiles[kh][:, kst],
                        lhsT=tt_sb[:, dpc, qc * P : (qc + 1) * P],
                        rhs=xt_sb[:, dpc, (kh * 2 + kst) * 512 : (kh * 2 + kst + 1) * 512],
                        start=(dpc == 0),
                        stop=(dpc == HO - 1),
                    )
        # exact per-row max -> negated bias for exp.  The DVE reduce chain is
        # emitted BEFORE the previous chunk's transpose so the in-order DVE
        # queue starts reducing the moment the score matmuls stop.
        m0 = smallb.tile([P, 1], F32, tag="m0", name="m0")
        m1 = smallb.tile([P, 1], F32, tag="m1", name="m1")
        nc.vector.reduce_max(m0[:], sc0[:], axis=mybir.AxisListType.XY)
        nc.vector.reduce_max(m1[:], sc1[:], axis=mybir.AxisListType.XY)
        negmax = smallb.tile([P, 1], F32, tag="negmax", name="negmax")
        nc.vector.tensor_tensor(negmax[:], m0[:], m1[:], mybir.AluOpType.max)
        nc.vector.tensor_scalar_mul(negmax[:], negmax[:], -1.0)
        acc0 = smallb.tile([P, 1], F32, tag="acc", name="acc0")
        acc1 = smallb.tile([P, 1], F32, tag="acc", name="acc1")
        nc.scalar.activation(
            exp_sb[:, 0:2],
            sc0[:],
            mybir.ActivationFunctionType.Exp,
            bias=negmax[:],
            scale=1.0,
            accum_out=acc0[:],
        )
        nc.scalar.activation(
            exp_sb[:, 2:4],
            sc1[:],
            mybir.ActivationFunctionType.Exp,
            bias=negmax[:],
            scale=1.0,
            accum_out=acc1[:],
        )
        den = smallb.tile([P, 1], F32, tag="den", name="den")
        nc.vector.tensor_add(out=den[:], in0=acc0[:], in1=acc1[:])
        nc.vector.reciprocal(rec_sb[:, qc : qc + 1], den[:])
        # XBAR transpose on the Activation HWDGE queue (SP carries the next
        # repeat's input preloads during this phase).  The LAST chunk's
        # transpose is deferred past the first D matmuls: phat dependency
        # tracking is writer-order based, so emitting it here would stall
        # D's cover tiles on qc=7's whole softmax chain.
        if qc < NQ - 1:
            nc.scalar.dma_start(
                phat_sb[:, :, qc * P : (qc + 1) * P], exp_sb[:], transpose=True
            )
        else:
            last_tp = exp_sb

    # ---- Phase D: U^T[d, q] = sum_k x[k, d] P^T[k, q] ----
    # Cover the last chunk's softmax+transpose window with D work that only
    # needs query columns 0:512 (phat from chunks 0..3): dt=0,1 low half.
    pth0 = ps_sc.tile([P, 2, 512], F32, tag="mm", name="pt_u_h0")
    for ks in range(SO):
        for j in range(2):
            nc.tensor.matmul(
                pth0[:, j],
                lhsT=xn_sb[:, ks, j * P : (j + 1) * P],
                rhs=phat_sb[:, ks, 0:512],
                start=(ks == 0),
                stop=(ks == SO - 1),
            )
    nc.scalar.dma_start(
        phat_sb[:, :, (NQ - 1) * P : NQ * P], last_tp[:], transpose=True
    )
    nc.vector.tensor_copy(out=ut_sb[:, 0, 0:512], in_=pth0[:, 0])
    nc.scalar.copy(out=ut_sb[:, 1, 0:512], in_=pth0[:, 1])
    pth1 = ps_sc.tile([P, 2, 512], F32, tag="mm", name="pt_u_h1")
    for ks in range(SO):
        for j in range(2):
            nc.tensor.matmul(
                pth1[:, j],
                lhsT=xn_sb[:, ks, j * P : (j + 1) * P],
                rhs=phat_sb[:, ks, 512:1024],
                start=(ks == 0),
                stop=(ks == SO - 1),
            )
    nc.vector.tensor_copy(out=ut_sb[:, 0, 512:1024], in_=pth1[:, 0])
    nc.scalar.copy(out=ut_sb[:, 1, 512:1024], in_=pth1[:, 1])
    for dt in range(2, HO):
        pt = ps_sc.tile([P, 2, 512], F32, tag="mm", name="pt_u")
        for ks in range(SO):
            for h in range(2):
                nc.tensor.matmul(
                    pt[:, h],
                    lhsT=xn_sb[:, ks, dt * P : (dt + 1) * P],
                    rhs=phat_sb[:, ks, h * 512 : (h + 1) * 512],
                    start=(ks == 0),
                    stop=(ks == SO - 1),
                )
        nc.vector.tensor_copy(out=ut_sb[:, dt, 0:512], in_=pt[:, 0])
        nc.scalar.copy(out=ut_sb[:, dt, 512:1024], in_=pt[:, 1])

    # ---- Phase E: y[q, f] = sum_d U^T[d, q] N[d, f] ----
    for qt in range(NQ):
        q0 = qt * P
        y_ps = ps_sc.tile([P, 2, 512], F32, tag="mm", name="y_ps")
        for dc in range(HO):
            for h in range(2):
                nc.tensor.matmul(
                    y_ps[:, h],
                    lhsT=ut_sb[:, dc, q0 : q0 + P],
                    rhs=n_sb[:, dc, h * 512 : (h + 1) * 512],
                    start=(dc == 0),
                    stop=(dc == HO - 1),
                )
        y_sb = ystage.tile([P, 2, 512], F16, tag="ystage", name="y_sb")
        # Softmax normalization happens here: per-partition (= per-query)
        # scale by 1/denom folded into the PSUM->SBUF copies.
        nc.vector.tensor_scalar_mul(y_sb[:, 0], y_ps[:, 0], rec_sb[:, qt : qt + 1])
        nc.scalar.activation(
            y_sb[:, 1],
            y_ps[:, 1],
            mybir.ActivationFunctionType.Copy,
            scale=rec_sb[:, qt : qt + 1],
        )
        # Output DMAs go on the Activation HWDGE queue so the next repeat's
        # input loads (SP queue) aren't serialized behind them.
        nc.scalar.dma_start(y_d[q0 : q0 + P, 0:512], y_sb[:, 0])
        nc.scalar.dma_start(y_d[q0 : q0 + P, 512:1024], y_sb[:, 1])


_cached_nc = None


def prep_in_maps(x, Wq, Wk, Wv, Wo):
    x = np.asarray(x, dtype=np.float32)
    Wq = np.asarray(Wq, dtype=np.float32)
    Wk = np.asarray(Wk, dtype=np.float32)
    Wv = np.asarray(Wv, dtype=np.float32)
    Wo = np.asarray(Wo, dtype=np.float32)

    # Host-side layout prep (no FLOPs): casts + transposes only.
    wq = Wq.astype(np.float16)
    wk = Wk.astype(np.float16)
    wv = Wv.astype(np.float16)
    wot = np.ascontiguousarray(Wo.T).astype(np.float16)

    in_maps = []
    for c in range(NCORES):
        b, h = divmod(c, 2)
        qlo = h * QH
        # roll keys so this core's queries are always at 0:QH
        xr = np.concatenate([x[b][qlo:], x[b][:qlo]], axis=0)
        xn = xr.astype(np.float16)
        xt = np.ascontiguousarray(xr.T).astype(np.float16)
        in_maps.append(
            {"xt": xt, "xn": xn, "wq": wq, "wk": wk, "wv": wv, "wot": wot}
        )
    return in_maps


def kernel(x, Wq, Wk, Wv, Wo):
    global _cached_nc
    from concourse.bass_utils import run_bass_kernel_spmd

    in_maps = prep_in_maps(x, Wq, Wk, Wv, Wo)

    if _cached_nc is None:
        _cached_nc = _build_program()
    res = run_bass_kernel_spmd(_cached_nc, in_maps, core_ids=list(range(NCORES)))

    out = np.empty((BATCH, SEQ, HIDDEN), dtype=np.float32)
    for c in range(NCORES):
        b, h = divmod(c, 2)
        out[b, h * QH : (h + 1) * QH, :] = res.results[c]["y"].astype(np.float32)
    return out


# revision 36
# speedup vs baseline: 23.0249x; 1.2404x over previous
"""Single-head dense attention (B=4, S=2048, H=1024) on 8 TRN2 NeuronCores.

Sharding: data-parallel, core c -> (batch b = c//2, query-half h = c%2).
Each core receives its batch's x in BOTH layouts (xT [H, S] and x [S, H],
fp16, rolled so its 1024 queries are always columns/rows 0:1024 — attention
is invariant to key order), plus Wq, Wk, Wv as stored [out, in] and Wo^T.

Weight-product reformulation: the reference is
    y = softmax(x Wq^T Wk x^T) x Wv^T Wo^T
so the four weights only enter through M = Wq^T Wk and N = Wv^T Wo^T
(both [H, H]). Computing M, N on device (2.15 GFLOP each) replaces the
K-projection and V-projection (4.3 GFLOP each at S=2048), cutting per-core
PE work from ~22.6 to ~17.7 GFLOP. Every matmul's lhsT/rhs falls in natural
layout: M <- (Wq, Wk), N <- (Wv, Wo^T), tT <- (M, xT), S <- (tT, xT),
U^T <- (x, P^T), y <- (U^T, N).

Per-core pipeline (fp16 inputs, fp32 PSUM accumulation; loops ordered so
each LDWEIGHTS is amortized over all matmuls sharing that lhsT):
  A. M[d,d'] and N[d,f]; chunked weight DMAs so compute starts early; Wv/WoT
     stream into the buffers Wq/Wk vacated (phase B covers the reload).
  B. tT[d',q] = sum_d M[d,d'] xT[d,q] over the core's 1024 query columns.
  C. Scores per 128-query chunk in natural [q,k] layout, each tT weight
     chunk amortized over 4 key tiles; exact softmax via per-row max (DVE
     reduce_max on score PSUM) as exp bias, exp to fp16, denominators free
     via ACT accum_out. The [q,k]->[k,q] transpose runs entirely on the DMA
     engines (XBAR transpose, one DMA per query chunk) — zero PE work.
  D. U^T[d,q] = sum_k x[k,d] P^T_raw[k,q] (P^T left unnormalized); the
     first two d-chunks are split by query half so their low-half matmuls
     cover the last chunk's softmax+transpose latency.
  E. y[q,f] = sum_d U^T[d,q] N[d,f]; softmax 1/denom folded into the
     PSUM->SBUF copies as a per-partition scale; fp16 y DMA'd on the
     second HWDGE queue.

Scheduling: all tile pools persist for the whole program (no alloc/release
boundaries between phases or repeats -> no PSUM/SBUF address-reuse stalls);
one shared 4-slot PSUM ring serves every accumulation phase; the next
repeat's input DMAs preload during this repeat's attention phases.
"""

from contextlib import ExitStack

import numpy as np

import bass_rust
import concourse.bass as bass
import concourse.mybir as mybir
import concourse.tile as tile
from concourse.vector_clock import ScopedClock
from concourse.masks import make_identity

HIDDEN = 1024
BATCH, SEQ = 4, 2048
P = 128
QH = 1024  # queries per core
NCORES = 8

F16 = mybir.dt.float16
F32 = mybir.dt.float32

# ---------------------------------------------------------------------------
# Workaround: walrus in this container encodes a limited number of sync-wait
# commands per instruction (1 for Matmult/Ldweights, ~4 for control insts).
# Split overflow waits onto same-engine NOPs inserted before the instruction,
# and split the Tile tail-drain waits onto sync-engine NOPs.
# ---------------------------------------------------------------------------
_MAXW = 1
_nop_ctr = [0]


def _patched_drain_and_barrier(self, tick_clock, wait_clock):
    nc = self.nc
    drain_inst = nc.sync.drain()
    wait_clock.add_sem_waits(
        drain_inst.ins, ScopedClock({None: tick_clock.global_clock})
    )
    si = drain_inst.ins.sync_info
    if si is not None and len(si.on_wait) > _MAXW:
        waits = list(si.on_wait)
        drain_inst.ins.sync_info = bass_rust.SyncInfo(
            on_wait=waits[:_MAXW], on_update=list(si.on_update)
        )
        for i in range(_MAXW, len(waits), _MAXW):
            nop = nc.sync.nop()
            nop.ins.sync_info = bass_rust.SyncInfo(
                on_wait=waits[i : i + _MAXW], on_update=[]
            )
    nc.all_engine_barrier()
    assert self.sems is not None
    popped = nc._tile_sem_poison_stack.pop()
    assert popped is self._sem_poison
    nc.clear_and_free_semaphores(list(self.sems.allocated().values()))
    nc.all_engine_barrier()


tile.TileContext._drain_and_barrier = _patched_drain_and_barrier


def _split_all_instruction_waits(nc):
    for f in nc.m.functions:
        for b in f.blocks:
            insts = b.instructions
            new_list = []
            changed = False
            for inst in insts:
                si = getattr(inst, "sync_info", None)
                if si is not None and len(si.on_wait) > _MAXW:
                    waits = list(si.on_wait)
                    keep = waits[-_MAXW:]
                    overflow = waits[:-_MAXW]
                    for j in range(0, len(overflow), _MAXW):
                        _nop_ctr[0] += 1
                        nop = mybir.InstNoOp(
                            name=f"I-waitsplit-{_nop_ctr[0]}",
                            engine=inst.engine,
                            bass_nofuse=True,
                            sync_info=bass_rust.SyncInfo(
                                on_wait=overflow[j : j + _MAXW], on_update=[]
                            ),
                        )
                        new_list.append(nop)
                    inst.sync_info = bass_rust.SyncInfo(
                        on_wait=keep, on_update=list(si.on_update)
                    )
                    changed = True
                new_list.append(inst)
            if changed:
                insts[:] = new_list


# ---------------------------------------------------------------------------
# Kernel program
# ---------------------------------------------------------------------------

def _build_program(repeat=1, tweak=0):
    nc = bass.Bass(
        "TRN2", target_bir_lowering=False, debug=False, num_devices=NCORES
    )
    xt_d = nc.dram_tensor("xt", [HIDDEN, SEQ], F16, kind="ExternalInput").ap()
    xn_d = nc.dram_tensor("xn", [SEQ, HIDDEN], F16, kind="ExternalInput").ap()
    wq_d = nc.dram_tensor("wq", [HIDDEN, HIDDEN], F16, kind="ExternalInput").ap()
    wk_d = nc.dram_tensor("wk", [HIDDEN, HIDDEN], F16, kind="ExternalInput").ap()
    wv_d = nc.dram_tensor("wv", [HIDDEN, HIDDEN], F16, kind="ExternalInput").ap()
    wot_d = nc.dram_tensor("wot", [HIDDEN, HIDDEN], F16, kind="ExternalInput").ap()
    y_d = nc.dram_tensor("y", [QH, HIDDEN], F16, kind="ExternalOutput").ap()

    HO = HIDDEN // P  # 8 chunks of the hidden dim
    SO = SEQ // P  # 16 chunks of the key seq dim

    with tile.TileContext(nc) as tc:
      for _t in range(tweak):
        nc.sync.nop()
      with ExitStack() as ctx:
        # All pools live for the whole program (every repeat): no pool
        # alloc/release boundaries between phases or repeats, so no PSUM /
        # SBUF address-reuse stalls at transitions.  PSUM: one shared ring
        # of 3x [P,2,512] slots (6 banks) for every accumulation phase +
        # 2x 1-bank transpose slots = 8 banks exactly.
        pools = {
            "pers": ctx.enter_context(tc.tile_pool(name="pers", bufs=1)),
            "pers2": ctx.enter_context(tc.tile_pool(name="pers2", bufs=1)),
            "attb": ctx.enter_context(tc.tile_pool(name="attb", bufs=2)),
            "smallb": ctx.enter_context(tc.tile_pool(name="smallb", bufs=6)),
            "ystage": ctx.enter_context(tc.tile_pool(name="ystage", bufs=2)),
            "wpool": ctx.enter_context(tc.tile_pool(name="wpool", bufs=1)),
            "ps": ctx.enter_context(tc.tile_pool(name="ps", bufs=4, space="PSUM")),
        }
        for _rep in range(repeat):
            _kernel_body(
                nc, tc, pools, xt_d, xn_d, wq_d, wk_d, wv_d, wot_d, y_d, HO, SO
            )

    _split_all_instruction_waits(nc)
    return nc


def _kernel_body(nc, tc, pools, xt_d, xn_d, wq_d, wk_d, wv_d, wot_d, y_d, HO, SO):
    pers = pools["pers"]
    xt_sb = pers.tile([P, HO, SEQ], F16, tag="xt")  # x^T [d, s] (queries 0:QH)
    xn_sb = pers.tile([P, SO, HIDDEN], F16, tag="xn")  # x [s, d]
    m_sb = pers.tile([P, HO, HIDDEN], F16, tag="m")  # M [d, d']
    n_sb = pers.tile([P, HO, HIDDEN], F16, tag="n")  # N [d, f]
    tt_sb = pers.tile([P, HO, QH], F16, tag="tt")  # t^T [d', q]
    pers2 = pools["pers2"]
    phat_sb = pers2.tile([P, SO, QH], F16, tag="phat")  # P^T [k, q] (raw exp)
    ut_sb = pers2.tile([P, HO, QH], F16, tag="ut")  # U^T [d, q] (unnormalized)
    rec_sb = pers2.tile([P, QH // P], F32, tag="recs")  # 1/denom per q chunk
    attb = pools["attb"]
    smallb = pools["smallb"]
    ystage = pools["ystage"]
    ps_mm = pools["ps"]

    # ---- Phase A/B: weight products M, N and t^T ----
    # Two weight buffers only: M consumes (Wq, Wk); afterwards Wv/WoT are
    # DMA'd into the same buffers for N (phase B covers that reload).
    wa_sb = pools["wpool"].tile([P, HO, HIDDEN], F16, tag="wa")
    wb_sb = pools["wpool"].tile([P, HO, HIDDEN], F16, tag="wb")
    wq_r = wq_d.rearrange("(o p) d -> p o d", p=P)
    wk_r = wk_d.rearrange("(o p) d -> p o d", p=P)
    wv_r = wv_d.rearrange("(o p) d -> p o d", p=P)
    wot_r = wot_d.rearrange("(o p) f -> p o f", p=P)
    xt_r = xt_d.rearrange("(o p) s -> p o s", p=P)
    xn_r = xn_d.rearrange("(o p) d -> p o d", p=P)
    # M's first PSUM group (dt=0) needs wq[:, :, 0:128] and wk chunk by
    # chunk — land those first so PE starts after ~0.5 MB of DMA.
    nc.sync.dma_start(wa_sb[:, :, 0:P], wq_r[:, :, 0:P])
    for dc in range(HO):
        nc.sync.dma_start(wb_sb[:, dc], wk_r[:, dc])
    for dt in range(1, HO):
        nc.sync.dma_start(
            wa_sb[:, :, dt * P : (dt + 1) * P], wq_r[:, :, dt * P : (dt + 1) * P]
        )
    for dc in range(HO):
        nc.sync.dma_start(xt_sb[:, dc, 0:QH], xt_r[:, dc, 0:QH])

    # M[d, d'] = sum_e Wq[e, d] Wk[e, d']
    for dt in range(HO):
        pt = ps_mm.tile([P, 2, 512], F32, tag="mm", name="pt_m")
        for ec in range(HO):
            for h in range(2):
                nc.tensor.matmul(
                    pt[:, h],
                    lhsT=wa_sb[:, ec, dt * P : (dt + 1) * P],
                    rhs=wb_sb[:, ec, h * 512 : (h + 1) * 512],
                    start=(ec == 0),
                    stop=(ec == HO - 1),
                )
        nc.vector.tensor_copy(out=m_sb[:, dt, 0:512], in_=pt[:, 0])
        nc.scalar.copy(out=m_sb[:, dt, 512:1024], in_=pt[:, 1])
    # Wv/WoT reload into the same buffers — emitted AFTER M's matmuls
    # (Tile dataflow is program-order), lands while B runs.
    for dc in range(HO):
        nc.sync.dma_start(wa_sb[:, dc], wv_r[:, dc])
        nc.sync.dma_start(wb_sb[:, dc], wot_r[:, dc])
    for dc in range(HO):
        nc.sync.dma_start(xt_sb[:, dc, QH:SEQ], xt_r[:, dc, QH:SEQ])
    for sc in range(SO):
        nc.sync.dma_start(xn_sb[:, sc], xn_r[:, sc])
    # t^T[d', q] = sum_d M[d, d'] xT[d, q]  (queries only).  Runs while
    # Wv/WoT stream into the weight buffers.
    for dpt in range(HO):
        pt = ps_mm.tile([P, 2, 512], F32, tag="mm", name="pt_t")
        for dc in range(HO):
            for h in range(2):
                nc.tensor.matmul(
                    pt[:, h],
                    lhsT=m_sb[:, dc, dpt * P : (dpt + 1) * P],
                    rhs=xt_sb[:, dc, h * 512 : (h + 1) * 512],
                    start=(dc == 0),
                    stop=(dc == HO - 1),
                )
        nc.vector.tensor_copy(out=tt_sb[:, dpt, 0:512], in_=pt[:, 0])
        nc.scalar.copy(out=tt_sb[:, dpt, 512:1024], in_=pt[:, 1])
    # N[d, f] = sum_e Wv[e, d] WoT[e, f] — also covers the tail latency
    # of tt_sb's last PSUM->SBUF copy before C.
    for dt in range(HO):
        pt = ps_mm.tile([P, 2, 512], F32, tag="mm", name="pt_n")
        for ec in range(HO):
            for h in range(2):
                nc.tensor.matmul(
                    pt[:, h],
                    lhsT=wa_sb[:, ec, dt * P : (dt + 1) * P],
                    rhs=wb_sb[:, ec, h * 512 : (h + 1) * 512],
                    start=(ec == 0),
                    stop=(ec == HO - 1),
                )
        nc.vector.tensor_copy(out=n_sb[:, dt, 0:512], in_=pt[:, 0])
        nc.scalar.copy(out=n_sb[:, dt, 512:1024], in_=pt[:, 1])

    # ---- Phase C: scores + softmax + XBAR-DMA transpose ----
    # The transpose off-loads to the DMA engines entirely: one XBAR DMA per
    # query chunk writes phat[:, ks, qc*P+p] = exp[q, ks*128+p] (verified
    # semantics).  Normalization (1/denom) is folded into phase E's
    # PSUM->SBUF copies as a per-partition scale, so PE does no transpose
    # work at all.
    ps_sc = ps_mm

    NQ = QH // P  # 8 query chunks of 128
    for qc in range(NQ):
        exp_sb = attb.tile([P, 4, 512], F16, tag="expP", name="exp_sb")
        sc0 = ps_sc.tile([P, 2, 512], F32, tag="mm", name="sc_ps0")
        sc1 = ps_sc.tile([P, 2, 512], F32, tag="mm", name="sc_ps1")
        sc_tiles = [sc0, sc1]
        # accumulation inner over dpc: each tT weight chunk amortized over
        # all 4 key tiles (one LDWEIGHTS per dpc)
        for dpc in range(HO):
            for kh in range(2):
                for kst in range(2):
                    nc.tensor.matmul(
                        sc_tiles[kh][:, kst],
                        lhsT=tt_sb[:, dpc, qc * P : (qc + 1) * P],
                        rhs=xt_sb[:, dpc, (kh * 2 + kst) * 512 : (kh * 2 + kst + 1) * 512],
                        start=(dpc == 0),
                        stop=(dpc == HO - 1),
                    )
        # exact per-row max -> negated bias for exp.  The DVE reduce chain is
        # emitted BEFORE the previous chunk's transpose so the in-order DVE
        # queue starts reducing the moment the score matmuls stop.
        m0 = smallb.tile([P, 1], F32, tag="m0", name="m0")
        m1 = smallb.tile([P, 1], F32, tag="m1", name="m1")
        nc.vector.reduce_max(m0[:], sc0[:], axis=mybir.AxisListType.XY)
        nc.vector.reduce_max(m1[:], sc1[:], axis=mybir.AxisListType.XY)
        negmax = smallb.tile([P, 1], F32, tag="negmax", name="negmax")
        nc.vector.tensor_tensor(negmax[:], m0[:], m1[:], mybir.AluOpType.max)
        nc.vector.tensor_scalar_mul(negmax[:], negmax[:], -1.0)
        acc0 = smallb.tile([P, 1], F32, tag="acc", name="acc0")
        acc1 = smallb.tile([P, 1], F32, tag="acc", name="acc1")
        nc.scalar.activation(
            exp_sb[:, 0:2],
            sc0[:],
            mybir.ActivationFunctionType.Exp,
            bias=negmax[:],
            scale=1.0,
            accum_out=acc0[:],
        )
        nc.scalar.activation(
            exp_sb[:, 2:4],
            sc1[:],
            mybir.ActivationFunctionType.Exp,
            bias=negmax[:],
            scale=1.0,
            accum_out=acc1[:],
        )
        den = smallb.tile([P, 1], F32, tag="den", name="den")
        nc.vector.tensor_add(out=den[:], in0=acc0[:], in1=acc1[:])
        nc.vector.reciprocal(rec_sb[:, qc : qc + 1], den[:])
        # XBAR transpose on the Activation HWDGE queue (SP carries the next
        # repeat's input preloads during this phase).  The LAST chunk's
        # transpose is deferred past the first D matmuls: phat dependency
        # tracking is writer-order based, so emitting it here would stall
        # D's cover tiles on qc=7's whole softmax chain.
        if qc < NQ - 1:
            nc.scalar.dma_start(
                phat_sb[:, :, qc * P : (qc + 1) * P], exp_sb[:], transpose=True
            )
        else:
            last_tp = exp_sb

    # ---- Phase D: U^T[d, q] = sum_k x[k, d] P^T[k, q] ----
    # Cover the last chunk's softmax+transpose window with D work that only
    # needs query columns 0:512 (phat from chunks 0..3): dt=0,1 low half.
    pth0 = ps_sc.tile([P, 2, 512], F32, tag="mm", name="pt_u_h0")
    for ks in range(SO):
        for j in range(2):
            nc.tensor.matmul(
                pth0[:, j],
                lhsT=xn_sb[:, ks, j * P : (j + 1) * P],
                rhs=phat_sb[:, ks, 0:512],
                start=(ks == 0),
                stop=(ks == SO - 1),
            )
    nc.scalar.dma_start(
        phat_sb[:, :, (NQ - 1) * P : NQ * P], last_tp[:], transpose=True
    )
    nc.vector.tensor_copy(out=ut_sb[:, 0, 0:512], in_=pth0[:, 0])
    nc.scalar.copy(out=ut_sb[:, 1, 0:512], in_=pth0[:, 1])
    pth1 = ps_sc.tile([P, 2, 512], F32, tag="mm", name="pt_u_h1")
    for ks in range(SO):
        for j in range(2):
            nc.tensor.matmul(
                pth1[:, j],
                lhsT=xn_sb[:, ks, j * P : (j + 1) * P],
                rhs=phat_sb[:, ks, 512:1024],
                start=(ks == 0),
                stop=(ks == SO - 1),
            )
    nc.vector.tensor_copy(out=ut_sb[:, 0, 512:1024], in_=pth1[:, 0])
    nc.scalar.copy(out=ut_sb[:, 1, 512:1024], in_=pth1[:, 1])
    for dt in range(2, HO):
        pt = ps_sc.tile([P, 2, 512], F32, tag="mm", name="pt_u")
        for ks in range(SO):
            for h in range(2):
                nc.tensor.matmul(
                    pt[:, h],
                    lhsT=xn_sb[:, ks, dt * P : (dt + 1) * P],
                    rhs=phat_sb[:, ks, h * 512 : (h + 1) * 512],
                    start=(ks == 0),
                    stop=(ks == SO - 1),
                )
        nc.vector.tensor_copy(out=ut_sb[:, dt, 0:512], in_=pt[:, 0])
        nc.scalar.copy(out=ut_sb[:, dt, 512:1024], in_=pt[:, 1])

    # ---- Phase E: y[q, f] = sum_d U^T[d, q] N[d, f] ----
    for qt in range(NQ):
        q0 = qt * P
        y_ps = ps_sc.tile([P, 2, 512], F32, tag="mm", name="y_ps")
        for dc in range(HO):
            for h in range(2):
                nc.tensor.matmul(
                    y_ps[:, h],
                    lhsT=ut_sb[:, dc, q0 : q0 + P],
                    rhs=n_sb[:, dc, h * 512 : (h + 1) * 512],
                    start=(dc == 0),
                    stop=(dc == HO - 1),
                )
        y_sb = ystage.tile([P, 2, 512], F16, tag="ystage", name="y_sb")
        # Softmax normalization happens here: per-partition (= per-query)
        # scale by 1/denom folded into the PSUM->SBUF copies.
        nc.vector.tensor_scalar_mul(y_sb[:, 0], y_ps[:, 0], rec_sb[:, qt : qt + 1])
        nc.scalar.activation(
            y_sb[:, 1],
            y_ps[:, 1],
            mybir.ActivationFunctionType.Copy,
            scale=rec_sb[:, qt : qt + 1],
        )
        # Output DMAs go on the Activation HWDGE queue so the next repeat's
        # input loads (SP queue) aren't serialized behind them.
        nc.scalar.dma_start(y_d[q0 : q0 + P, 0:512], y_sb[:, 0])
        nc.scalar.dma_start(y_d[q0 : q0 + P, 512:1024], y_sb[:, 1])


_cached_nc = None


def prep_in_maps(x, Wq, Wk, Wv, Wo):
    x = np.asarray(x, dtype=np.float32)
    Wq = np.asarray(Wq, dtype=np.float32)
    Wk = np.asarray(Wk, dtype=np.float32)
    Wv = np.asarray(Wv, dtype=np.float32)
    Wo = np.asarray(Wo, dtype=np.float32)

    # Host-side layout prep (no FLOPs): casts + transposes only.
    wq = Wq.astype(np.float16)
    wk = Wk.astype(np.float16)
    wv = Wv.astype(np.float16)
    wot = np.ascontiguousarray(Wo.T).astype(np.float16)

    in_maps = []
    for c in range(NCORES):
        b, h = divmod(c, 2)
        qlo = h * QH
        # roll keys so this core's queries are always at 0:QH
        xr = np.concatenate([x[b][qlo:], x[b][:qlo]], axis=0)
        xn = xr.astype(np.float16)
        xt = np.ascontiguousarray(xr.T).astype(np.float16)
        in_maps.append(
            {"xt": xt, "xn": xn, "wq": wq, "wk": wk, "wv": wv, "wot": wot}
        )
    return in_maps


def kernel(x, Wq, Wk, Wv, Wo):
    global _cached_nc
    from concourse.bass_utils import run_bass_kernel_spmd

    in_maps = prep_in_maps(x, Wq, Wk, Wv, Wo)

    if _cached_nc is None:
        _cached_nc = _build_program()
    res = run_bass_kernel_spmd(_cached_nc, in_maps, core_ids=list(range(NCORES)))

    out = np.empty((BATCH, SEQ, HIDDEN), dtype=np.float32)
    for c in range(NCORES):
        b, h = divmod(c, 2)
        out[b, h * QH : (h + 1) * QH, :] = res.results[c]["y"].astype(np.float32)
    return out
